# revision 15
# baseline (speedup 1.0000x reference)
"""DeepseekV4 hash-router MoE routing kernel for Trainium2 (8 NeuronCores).

Strategy (data-parallel over tokens, per sharding hint):
  - Shard the flattened token dim N=16384 across 8 cores (2048 tokens each).
  - Host prep per kernel() call: hidden is cast to bf16 and shipped
    PRE-TRANSPOSED and block-packed so the device needs no PE transposes
    and every DMA line is long and contiguous; the gate weight ships as
    bf16 in [d-in-block, (block, expert)] layout; the per-token one-hot
    routing rows (a pure function of token_ids and tid2eid, no gate math)
    are gathered on the host and shipped as a [128, NT*E] u8 mask.
  - Per core on device: 16 token tiles, each a [128d,128t]x[128d,256e]
    x16-block PSUM-accumulated bf16 matmul; sqrt(softplus(x)) =
    exp(0.5*ln(ln(exp(x)+1))) on the scalar engine (single activation
    table, batched GRP tiles per pass); DVE mask-multiply+row-reduce
    against the one-hot rows, reciprocal, scale; probs written out as
    bf16 (host upcasts to f32).
  - routing_map == the one-hot mask (device-independent); the host
    assembles it directly from the same array it shipped to the device.
  - No cross-core communication; outputs are concatenated on the host.
"""

import numpy as np
import ml_dtypes

import concourse.bass as bass
import concourse.mybir as mybir
import concourse.tile as tile
from concourse import bacc
from concourse.bass_utils import run_bass_kernel_spmd

# Problem shape (hardcoded; kernel.py must be self-contained).
B, S, D = 4, 4096, 2048
E, K, V = 256, 8, 128000
SCALE = 2.5
NCORES = 8
N = B * S            # 16384 flattened tokens
NLOC = N // NCORES   # 2048 tokens per core
P = 128              # partitions
NT = NLOC // P       # 16 token tiles per core
ND = D // P          # 16 contraction blocks

F32 = mybir.dt.float32
BF16 = mybir.dt.bfloat16
F8E4 = mybir.dt.float8e4
U8 = mybir.dt.uint8
AF = mybir.ActivationFunctionType
OP = mybir.AluOpType
PM = mybir.MatmulPerfMode

BF = ml_dtypes.bfloat16
F8 = ml_dtypes.float8_e4m3

# fp8 path: hidden/weight quantize to e4m3; weight pre-scaled by WSCALE on
# the host so its ~N(0, 0.02^2) values clear the e4m3 subnormal range, and
# the first activation un-scales via its free affine (exp(x/WSCALE)).
MM_DT = "fp8"       # "bf16" | "fp8"
DOUBLE_ROW = True   # fp8 only: 2 contraction rows per PE cell
WSCALE = 32.0

_CACHE: dict = {}


def _unify_act_tables(arch: str):
    """Make Exp and Ln resolve only to natural_log_exp_and_others.

    bacc's greedy table chooser otherwise picks exp_and_others for Exp and
    natural_log for Ln, inserting ~2 ACT_TABLE_LOADs (~2.7us each) per
    activation group.  Removing Exp/Ln from every other candidate set forces
    a single resident table for the whole kernel (the combined set really
    does contain both functions), so the load hoists out of the loop.
    """
    import concourse.hw_specs as hw_specs

    tabs = hw_specs.get_activation_tables(arch)  # functools.cache: shared dict
    for name, fns in tabs.items():
        if name != "natural_log_exp_and_others":
            fns.discard(AF.Exp)
            fns.discard(AF.Ln)


def _build(
    reps: int = 1,
    grp: int = 4,
    cht: int = 4,
    hin_bufs: int = 3,
    mm_bufs: int = 4,
    mm_dt: str | None = None,
    double_row: bool | None = None,
    dr_mode: str = "dr",  # "dr" | "drsw" (DoubleRowSwInterleave)
    no_pe: bool = False,
    no_act: bool = False,
    no_dve: bool = False,
    no_hid: bool = False,
    no_out: bool = False,
):
    if mm_dt is None:
        mm_dt = MM_DT
    if double_row is None:
        double_row = DOUBLE_ROW and mm_dt == "fp8"
    MDT = F8E4 if mm_dt == "fp8" else BF16
    exp_scale = (1.0 / WSCALE) if mm_dt == "fp8" else 1.0
    nch = NT // cht  # hidden chunks per rep
    nc = bacc.Bacc(
        "TRN2", target_bir_lowering=False, debug=False, enable_asserts=False
    )
    _unify_act_tables(nc.m.arch)

    # row p of hidT holds, for (j, b, t): hidden[j*128 + t, b*128 + p]
    hidT = nc.dram_tensor("hidT", [P, NT * ND * P], MDT, kind="ExternalInput")
    # row p of wt holds, for (b, e): weight[e, b*128 + p]
    wt = nc.dram_tensor("wt", [P, ND * E], MDT, kind="ExternalInput")
    # row t of oh holds, for (j, e): onehot[j*128 + t, e]
    oh = nc.dram_tensor("oh", [P, NT * E], U8, kind="ExternalInput")
    probs = nc.dram_tensor("probs", [NLOC, E], BF16, kind="ExternalOutput")
    # col-grouped view: row p, cols (j, e) -> token j*128+p
    probs_r = probs.ap().rearrange("(j p) e -> p j e", p=P)

    with tile.TileContext(nc) as tc:
        with (
            tc.tile_pool(name="const", bufs=1) as cpool,
            tc.tile_pool(name="hin", bufs=hin_bufs) as hin_pool,
            tc.tile_pool(name="ohp", bufs=2) as oh_pool,
            tc.tile_pool(name="mm_ps", bufs=mm_bufs, space="PSUM") as mm_psum,
            tc.tile_pool(name="sc", bufs=2) as sc_pool,
            tc.tile_pool(name="nrm", bufs=3) as nrm_pool,
            tc.tile_pool(name="outp", bufs=3) as out_pool,
        ):
            wt_sb = cpool.tile([P, ND * E], MDT)
            nc.sync.dma_start(wt_sb[:], wt.ap())

            def emit_group_tail(rep, g_idx, ex_all, oh_all):
                # scores = sqrt(softplus(x)) = exp(0.5*ln(ln(exp(x)+1))):
                # Exp/Ln only, so every activation stays in the single
                # natural_log_exp_and_others table. Logits are ~N(0,1) so
                # exp never overflows.
                sp = sc_pool.tile([P, grp * E], F32, tag="sp", name=f"sp_r{rep}g{g_idx}")
                lsp = sc_pool.tile([P, grp * E], F32, tag="lsp", name=f"lsp_r{rep}g{g_idx}")
                sc = sc_pool.tile([P, grp * E], F32, tag="sc", name=f"sc_r{rep}g{g_idx}")
                if not no_act:
                    nc.scalar.activation(sp[:], ex_all[:], AF.Ln, bias=1.0)
                    nc.scalar.activation(lsp[:], sp[:], AF.Ln)
                    nc.scalar.activation(sc[:], lsp[:], AF.Exp, scale=0.5)
                else:
                    sc = ex_all

                probs_g = out_pool.tile(
                    [P, grp * E], BF16, tag="probs_g", name=f"pg_r{rep}g{g_idx}"
                )
                for q in range(grp):
                    j = g_idx * grp + q
                    probs_t = probs_g[:, q * E : (q + 1) * E]
                    if not no_dve:
                        # masked scores + their per-token sum in one DVE op
                        oh_t = oh_all[:, j * E : (j + 1) * E]
                        msc = nrm_pool.tile([P, E], F32, tag="msc", name=f"ms_r{rep}j{j}")
                        den = nrm_pool.tile([P, 1], F32, tag="den", name=f"dn_r{rep}j{j}")
                        nc.vector.scalar_tensor_tensor(
                            out=msc[:],
                            in0=sc[:, q * E : (q + 1) * E],
                            scalar=0.0,
                            in1=oh_t,
                            op0=OP.bypass,
                            op1=OP.mult,
                            accum_out=den[:],
                        )
                        rden = nrm_pool.tile([P, 1], F32, tag="rden", name=f"rd_r{rep}j{j}")
                        nc.vector.reciprocal(rden[:], den[:])
                        nc.vector.tensor_scalar(
                            probs_t,
                            msc[:],
                            rden[:, 0:1],
                            SCALE,
                            op0=OP.mult,
                            op1=OP.mult,
                        )
                    else:
                        nc.vector.tensor_copy(probs_t, sc[:, q * E : (q + 1) * E])
                # one batched output DMA per activation group
                if not no_out:
                    nc.sync.dma_start(
                        probs_r[:, g_idx * grp : (g_idx + 1) * grp, :],
                        probs_g[:].rearrange("p (q e) -> p q e", q=grp),
                    )

            for rep in range(reps):
                oh_all = oh_pool.tile([P, NT * E], U8, tag="oh", name=f"oh_r{rep}")
                if not no_dve:
                    nc.sync.dma_start(oh_all[:], oh.ap())
                ex_all = None
                for c in range(nch):
                    hch = hin_pool.tile(
                        [P, cht * ND * P], MDT, tag="hid", name=f"h_r{rep}c{c}"
                    )
                    if not no_hid:
                        nc.sync.dma_start(
                            hch[:],
                            hidT.ap()[:, c * cht * ND * P : (c + 1) * cht * ND * P],
                        )
                    for jj in range(cht):
                        j = c * cht + jj
                        q = j % grp
                        if q == 0:
                            ex_all = sc_pool.tile(
                                [P, grp * E], F32, tag="ex", name=f"ex_r{rep}g{j // grp}"
                            )
                        lg = mm_psum.tile([P, E], F32, tag="lg", name=f"lg_r{rep}j{j}")
                        if no_pe:
                            nc.vector.memset(lg[:], 0.5)
                        elif double_row:
                            # 2 packed fp8 contraction rows per PE cell:
                            # lhsT [ki, ko=2, t], rhs [ki, ko=2, e] where
                            # d = (2*b2 + ko)*128 + ki on both sides (host
                            # packing order is already (b2, ko)-major).
                            for b2 in range(ND // 2):
                                k = jj * ND + 2 * b2
                                nc.tensor.matmul(
                                    lg[:],
                                    lhsT=hch[
                                        :, k * P : (k + 2) * P
                                    ].rearrange("p (ko t) -> p ko t", ko=2),
                                    rhs=wt_sb[
                                        :, 2 * b2 * E : (2 * b2 + 2) * E
                                    ].rearrange("p (ko e) -> p ko e", ko=2),
                                    start=(b2 == 0),
                                    stop=(b2 == ND // 2 - 1),
                                    perf_mode=(
                                        PM.DoubleRowSwInterleave
                                        if dr_mode == "drsw"
                                        else PM.DoubleRow
                                    ),
                                )
                        else:
                            for b in range(ND):
                                k = jj * ND + b
                                nc.tensor.matmul(
                                    lg[:],
                                    lhsT=hch[:, k * P : (k + 1) * P],
                                    rhs=wt_sb[:, b * E : (b + 1) * E],
                                    start=(b == 0),
                                    stop=(b == ND - 1),
                                )
                        # Exp doubles as the PSUM->SBUF move (per tile).
                        nc.scalar.activation(
                            ex_all[:, q * E : (q + 1) * E], lg[:], AF.Exp,
                            scale=exp_scale,
                        )
                        if q == grp - 1:
                            emit_group_tail(rep, j // grp, ex_all, oh_all)

    nc.compile()
    return nc


def _get_nc():
    if "nc" not in _CACHE:
        _CACHE["nc"] = _build()
    return _CACHE["nc"]


def prepare_in_maps(hidden, tids, weight, tid2eid, mm_dt=None):
    """hidden [N, D] f32, tids [N] i64/i32, weight [E, D] f32, tid2eid [V, K].

    Returns (in_maps, ohr) where ohr is the [N, E] u8 one-hot routing mask
    (shared with the device; also the routing_map output).
    """
    if mm_dt is None:
        mm_dt = MM_DT
    mdt = F8 if mm_dt == "fp8" else BF
    wmul = WSCALE if mm_dt == "fp8" else 1.0
    hid_bf = np.ascontiguousarray(hidden).astype(mdt)  # [N, D]
    wt_p = (
        (np.ascontiguousarray(np.asarray(weight, np.float32).T) * wmul)  # [D, E]
        .reshape(ND, P, E)
        .transpose(1, 0, 2)
        .reshape(P, ND * E)
        .astype(mdt)
    )
    t2e8 = np.asarray(tid2eid, np.int64)[np.asarray(tids, np.int64)]  # [N, K]
    ohr = np.zeros((N, E), np.uint8)
    ohr[np.arange(N)[:, None], t2e8] = 1

    in_maps = []
    for c in range(NCORES):
        n0 = c * NLOC
        hc = (
            hid_bf[n0 : n0 + NLOC]
            .reshape(NT, P, ND, P)          # (j, t, b, p)
            .transpose(3, 0, 2, 1)          # (p, j, b, t)
            .reshape(P, NT * ND * P)
        )
        ohc = (
            ohr[n0 : n0 + NLOC]
            .reshape(NT, P, E)              # (j, t, e)
            .transpose(1, 0, 2)             # (t, j, e)
            .reshape(P, NT * E)
        )
        in_maps.append(
            {
                "hidT": np.ascontiguousarray(hc),
                "wt": wt_p,
                "oh": np.ascontiguousarray(ohc),
            }
        )
    return in_maps, ohr


def kernel(hidden, token_ids, weight, tid2eid):
    hidden = np.asarray(hidden, dtype=np.float32).reshape(N, D)
    tids = np.asarray(token_ids).reshape(N)

    nc = _get_nc()
    in_maps, ohr = prepare_in_maps(hidden, tids, weight, tid2eid)
    res = run_bass_kernel_spmd(nc, in_maps, core_ids=list(range(NCORES)))
    _CACHE["last_results"] = res

    probs = np.concatenate([r["probs"] for r in res.results], axis=0).astype(
        np.float32
    )
    rmap = ohr.astype(bool)
    return probs, rmap
